# revision 20
# baseline (speedup 1.0000x reference)
"""TRN2 Bass kernel for nn_Attention_11252814315826.

out[b,h,s,:] = softmax(Q[b,h] @ K^T[b,h] / 8 + addr(mask)) @ V[b,h]
with the additive mask on the QUERY dim: for mask[b,s]==0 the reference's
-1e12 row offset makes softmax exactly uniform, so out = colmean(V[b,h]).

Strategy (v3): shard the 32 (b,h) pairs 4-per-core across 8 NeuronCores.
Host-side: compact query rows to the mask==1 subset, pre-transpose to
Q^T [128, SP] fp16 with rows 64-127 a duplicate of 0-63, and pack K^T as
[128, 8, 128] fp16 holding key-block pairs (even t in partitions 0-63,
odd t in 64-127).

Device per pair: QK^T runs as TWO CONCURRENT row-tiled 64-contract
matmuls (tile_position rows 0-63 / 64-127) writing the two bank-halves
of one [128, 1024] PSUM tile. This both doubles QK throughput and -- the
key discovery -- keeps the PE HAM un-throttled: matmuls that only drive
64 of the 128 contract rows never reach the "busy" activity threshold
and the PE stays clamped at 1.2 GHz; full-row activity runs at 2.4 GHz.
A ~5us burst of full-contract warmup matmuls during the initial DMA fill
triggers the un-throttle before real work starts.

exp uses the bitcast fast-exp (i16 = score*184.66 + const, reinterpreted
as fp16 == 2^(x*log2e) with linear mantissa interp; the +-3% sawtooth
cancels through softmax normalization only if EVERY key block uses the
identical formula, so both engines run the same math). One fused op per
slot covers both PSUM halves (FD=1024) to amortize the per-op overhead:
wide slots alternate ACT (Copy activation w/ scale+bias) and DVE
(tensor_scalar mult+add); narrow tail slots go to the DVE, per-chunk
epilogues to the ACT, balancing both engines just under the PE pace.

PV accumulates [V|1]^T @ E in PSUM giving numerator and denominator
together. The epilogue copies raw [num|den] to fp16 SBUF and DMAs it
out; f32 divide and [d,s]->[s,d] transpose happen on the host.
"""

import os
import sys

for _p in (
    "/root/.axon_site",
    "/root/.axon_site/_ro/trn_rl_repo",
    "/root/.axon_site/_ro/pypackages",
    "/opt/trn_rl_repo",
):
    if os.path.isdir(_p) and _p not in sys.path:
        sys.path.append(_p)

from concourse.bass_utils import run_bass_kernel_spmd

import numpy as np

import concourse.bacc as bacc
import concourse.tile as tile
import concourse.mybir as mybir

F32 = mybir.dt.float32
F16 = mybir.dt.float16
I16 = mybir.dt.int16

LOG2E = 1.4426950408889634
S0 = 3.0  # exponent shift: exp(x/8 - S0); cancels in softmax, keeps fp16 range
FE_SCALE = 0.125 * 1024 * LOG2E          # 184.66496...
FE_BIAS = 15 * 1024 - S0 * 1024 * LOG2E - 44.0


def _chunk_plan(SP):
    """Split SP query columns into chunks of width <=512 (PSUM bank limit)."""
    chunks = []
    s0 = 0
    while s0 < SP:
        w = min(512, SP - s0)
        chunks.append((s0, w))
        s0 += w
    return chunks


def build_attention_nc(NP=4, SP=1056, S=2048, D=64):
    assert S % 256 == 0 and D == 64 and SP % 32 == 0
    NT = S // 128   # 16 key blocks of 128
    NJ = NT // 2    # 8 packed key-block pairs
    chunks = _chunk_plan(SP)
    NCH = len(chunks)

    nc = bacc.Bacc("TRN2", target_bir_lowering=False, debug=False)

    qt = nc.dram_tensor("qt", [NP, 128, SP], F16, kind="ExternalInput")
    kt = nc.dram_tensor("kt", [NP, 128, NJ, 128], F16, kind="ExternalInput")
    # v pre-arranged on host to the device layout [128, NT, 65] so the
    # load is one big contiguous-row DMA (strided-gather descriptors were
    # serializing the Sync engine's DMA dispatch).
    v = nc.dram_tensor("v", [NP, 128, NT, D + 1], F16, kind="ExternalInput")
    o = nc.dram_tensor("o", [NP, D + 1, NCH, 512], F16, kind="ExternalOutput")

    dma = nc.sync

    debug = os.environ.get("KERNEL_DEBUG", "0") == "1"
    if debug:
        dbg_qk = nc.dram_tensor("dbg_qk", [NJ, 128, 2, 512], F32, kind="ExternalOutput")
        dbg_exp = nc.dram_tensor("dbg_exp", [NJ, 128, 2, 512], F16, kind="ExternalOutput")

    ctxs = {}

    with tile.TileContext(nc) as tc:
        with (
            tc.tile_pool(name="const", bufs=1) as const_pool,
            tc.tile_pool(name="kt", bufs=1) as kt_pool,
            tc.tile_pool(name="qt", bufs=1) as qt_pool,
            tc.tile_pool(name="v", bufs=1) as v_pool,
            tc.tile_pool(name="exp", bufs=6) as exp_pool,
            tc.tile_pool(name="osb", bufs=4) as osb_pool,
            tc.tile_pool(name="qkps", bufs=3, space="PSUM") as qk_psum,
            tc.tile_pool(name="pvps", bufs=2, space="PSUM") as pv_psum,
        ):
            ident = const_pool.tile([128, 128], F16)
            nc.vector.memset(ident[:], 0.0)

            def pe_warmup(n=int(os.environ.get("KERNEL_WARM", "48"))):
                # Full-contract (128-row) matmuls during the initial DMA
                # fill: HAM only counts full-row activity toward the
                # "busy" un-throttle threshold, so these flip the PE to
                # 2.4 GHz before the real stream begins. The zeros tile
                # comes from a DVE memset, not a DMA, so warmup starts at
                # t~0 instead of waiting out the DMA-queue preamble.
                warm_ps = qk_psum.tile([128, 2, 512], F32, tag="qkp", name="warm")
                for i in range(n):
                    nc.tensor.matmul(
                        warm_ps[:, 0, 0:128],
                        ident[:, :],
                        ident[:, :],
                        start=True,
                        stop=True,
                        skip_group_check=True,
                    )
                warm_out = const_pool.tile([128, 8], F32, tag="warmout")
                nc.vector.tensor_copy(warm_out[:], warm_ps[:, 0, 0:8])

            def pair_prologue(p):
                # input DMAs issued once; inputs stay resident in SBUF.
                # One big contiguous DMA per tensor: Sync-engine dispatch
                # (~0.8us per dma_start) was the hidden serial resource.
                qt_sb = qt_pool.tile([128, SP], F16, tag=f"qt{p}", name=f"qt{p}")
                dma.dma_start(qt_sb[:], qt.ap()[p])
                kt_sb = kt_pool.tile([128, NJ, 128], F16, tag=f"kt{p}", name=f"kt{p}")
                dma.dma_start(kt_sb[:], kt.ap()[p])
                v_sb = v_pool.tile([128, NT, D + 1], F16, tag=f"v{p}", name=f"v{p}")
                dma.dma_start(v_sb[:], v.ap()[p])
                ctxs[p] = dict(kt=kt_sb, qt=qt_sb, v=v_sb)

            ballast = os.environ.get("KERNEL_BALLAST", "1") == "1"

            def emit_qk(p, s0, sw, j):
                # two concurrent row-tiled 64-contract matmuls (rows 0-63
                # and 64-127) -> the two bank-halves of one PSUM tile.
                cx = ctxs[p]
                qk_ps = qk_psum.tile([128, 2, 512], F32, tag="qkp")
                nc.tensor.matmul(
                    qk_ps[:, 0, 0:sw],
                    cx["kt"][0:64, j, :],
                    cx["qt"][0:64, s0 : s0 + sw],
                    start=True,
                    stop=True,
                    skip_group_check=True,
                )
                nc.tensor.matmul(
                    qk_ps[:, 1, 0:sw],
                    cx["kt"][64:128, j, :],
                    cx["qt"][64:128, s0 : s0 + sw],
                    start=True,
                    stop=True,
                    skip_group_check=True,
                )
                if ballast and sw <= 128:
                    # narrow tail slots leave the PE mostly idle; HAM then
                    # reads low full-row activity across the 3.4us window and
                    # re-throttles the clock at every pair boundary. A dummy
                    # full-contract matmul into unused PSUM columns of this
                    # same tile keeps the activity monitor fed.
                    nc.tensor.matmul(
                        qk_ps[:, 0, 256:512],
                        ident[:, :],
                        cx["qt"][:, 0:256],
                        start=True,
                        stop=True,
                        skip_group_check=True,
                    )
                return qk_ps

            exp_rot = [0]

            def emit_exp(p, sw, qk_ps, narrow):
                # one fused op over both halves; identical fast-exp math
                # on either engine (sawtooth must match across all key
                # blocks to cancel in normalization).
                # split mode: the slot's two halves go to BOTH engines in
                # parallel -> per-slot exp latency (~690ns) fits inside one
                # PE slot, so lagged PV matmuls never wait on exp. fused
                # mode: one FD=2*sw op on alternating engines -> lower
                # total engine time (overhead amortized) but ~2-slot
                # latency; needs a deeper PV lag.
                exp_sb = exp_pool.tile([128, 2, 512], F16, tag="exp")
                if os.environ.get("KERNEL_EXPMODE", "split") == "fused":
                    if narrow:
                        eng = 1  # keep wide-slot engine parity undisturbed
                    else:
                        eng = exp_rot[0] % 2
                        exp_rot[0] += 1
                    plan = [(qk_ps[:, :, 0:sw], exp_sb[:, :, 0:sw].bitcast(I16), eng)]
                else:
                    plan = [
                        (qk_ps[:, h, 0:sw], exp_sb[:, h, 0:sw].bitcast(I16), h)
                        for h in range(2)
                    ]
                for src, dst, eng in plan:
                    if eng == 0:
                        nc.scalar.activation(
                            dst,
                            src,
                            mybir.ActivationFunctionType.Copy,
                            scale=FE_SCALE,
                            bias=FE_BIAS,
                        )
                    else:
                        nc.vector.tensor_scalar(
                            dst,
                            src,
                            FE_SCALE,
                            FE_BIAS,
                            op0=mybir.AluOpType.mult,
                            op1=mybir.AluOpType.add,
                        )
                return exp_sb

            def make_pv(p, sw, j, half, exp_sb, pv_ps):
                t = 2 * j + half

                def emit():
                    v_sb = ctxs[p]["v"]
                    nc.tensor.matmul(
                        pv_ps[:, 0:sw],
                        v_sb[:, t, :],
                        exp_sb[:, half, 0:sw],
                        start=(t == 0),
                        stop=(t == NT - 1),
                        skip_group_check=True,
                    )

                return emit

            def make_epilogue(p, ci, sw, pv_ps, o_sb, last):
                # raw [num | den] into the pair's staging tile; one DMA per
                # pair after its last chunk. Normalization and the
                # [d, s] -> [s, d] transpose happen on the host.
                def emit():
                    if exp_rot[0] % 2 == 0:
                        nc.scalar.activation(
                            o_sb[:, ci, 0:sw],
                            pv_ps[:, 0:sw],
                            mybir.ActivationFunctionType.Copy,
                            scale=1.0,
                        )
                    else:
                        nc.vector.tensor_copy(o_sb[:, ci, 0:sw], pv_ps[:, 0:sw])
                    exp_rot[0] += 1
                    if last:
                        dma.dma_start(o.ap()[p], o_sb[:])

                return emit

            # ---- slot-pipelined emission -----------------------------------
            # Per slot (key-block pair j): packed QK -> fused exp -> two PV
            # matmuls with lag so exp can finish. Epilogues deferred 3 slots.
            def emit_body():
                step = [0]
                pvq = []      # deferred (chunk_uid, fn) PV emitters (lag in slots)
                delayed = []  # (due_step, chunk_uid, fn) epilogues
                lag = 2 * int(os.environ.get("KERNEL_PVLAG", "2"))

                def flush_chunk(uid):
                    # emit any of this chunk's PV matmuls still queued BEFORE
                    # its epilogue reads the PSUM accumulator (front entries
                    # are oldest, so pop from the front).
                    while pvq and pvq[0][0] <= uid:
                        pvq.pop(0)[1]()

                def tick():
                    step[0] += 1
                    for due, uid, fn in [d for d in delayed if d[0] <= step[0]]:
                        delayed.remove((due, uid, fn))
                        flush_chunk(uid)
                        fn()
                    while len(pvq) >= lag:
                        pvq.pop(0)[1]()

                uid = 0
                for p in range(NP):
                    o_sb = osb_pool.tile([D + 1, NCH, 512], F16, tag="osb")
                    for ci, (s0, sw) in enumerate(chunks):
                        uid += 1
                        pv_ps = pv_psum.tile([D + 1, sw], F32, tag="pvp")
                        narrow = sw <= 128
                        for j in range(NJ):
                            tick()
                            qk_ps = emit_qk(p, s0, sw, j)
                            if debug and p == 0 and ci == 0:
                                dqk = osb_pool.tile([128, 2, 512], F32, tag=f"dqk{j}")
                                nc.vector.tensor_copy(dqk[:], qk_ps[:, :, :])
                                dma.dma_start(dbg_qk.ap()[j], dqk[:])
                            exp_sb = emit_exp(p, sw, qk_ps, narrow)
                            if debug and p == 0 and ci == 0:
                                dma.dma_start(dbg_exp.ap()[j], exp_sb[:, :, :])
                            pvq.append((uid, make_pv(p, sw, j, 0, exp_sb, pv_ps)))
                            pvq.append((uid, make_pv(p, sw, j, 1, exp_sb, pv_ps)))
                        delayed.append(
                            (
                                step[0] + 3,
                                uid,
                                make_epilogue(
                                    p, ci, sw, pv_ps, o_sb, ci == NCH - 1
                                ),
                            )
                        )
                while pvq:
                    pvq.pop(0)[1]()
                for _, uid, fn in delayed:
                    fn()

            for p in range(NP):
                pair_prologue(p)
            pe_warmup()
            emit_body()

    nc.compile()
    return nc


B, H = 2, 16
S, D = 2048, 64
N_CORES = 8
PAIRS_PER_CORE = (B * H) // N_CORES  # 4

_NC_CACHE = {}
last_results = None


def _install_profile_hook():
    """Wire up the axon NTFF profiling hook if the image's antenv lacks it."""
    import types

    try:
        import antenv.axon_hooks  # noqa: F401

        return
    except ImportError:
        pass
    try:
        from trn_agent_boot.trn_boot import _ntff_profile_via_ctypes

        hook = _ntff_profile_via_ctypes("/opt/axon/libaxon_pjrt.so")
    except Exception:
        hook = None
    mod = types.ModuleType("antenv.axon_hooks")
    mod._hook = hook
    mod.get_axon_ntff_profile_hook = lambda: mod._hook
    mod.set_axon_ntff_profile_hook = lambda h: setattr(mod, "_hook", h)
    sys.modules["antenv.axon_hooks"] = mod
    import antenv

    antenv.axon_hooks = mod
    import concourse.bass_utils as _bu

    _bu.upload_artifacts = lambda tmpdir: "local://" + tmpdir


def _plan(mask):
    idx = [np.nonzero(mask[b] != 0)[0] for b in range(B)]
    cnt = [len(ix) for ix in idx]
    need = max(c + (1 if c < S else 0) for c in cnt)
    SP = max(128, -(-need // 32) * 32)
    return idx, cnt, SP


def build_in_maps(query, key, value, idx, cnt, SP):
    NJ = S // 256
    NT = S // 128
    in_maps = []
    for c in range(N_CORES):
        qs = np.zeros((PAIRS_PER_CORE, 128, SP), dtype=np.float16)
        ks = np.empty((PAIRS_PER_CORE, 128, NJ, 128), dtype=np.float16)
        vs = np.empty((PAIRS_PER_CORE, 128, NT, D + 1), dtype=np.float16)
        for i in range(PAIRS_PER_CORE):
            pair = c * PAIRS_PER_CORE + i
            b, h = pair // H, pair % H
            qT = query[b, h, idx[b]].T.astype(np.float16)  # [D, cnt]
            qs[i, :D, : cnt[b]] = qT
            qs[i, D:, : cnt[b]] = qT
            kb = key[b, h].astype(np.float16).reshape(D, NJ, 2, 128)
            ks[i, :D] = kb[:, :, 0, :]
            ks[i, D:] = kb[:, :, 1, :]
            vf = np.ones((S, D + 1), dtype=np.float16)
            vf[:, :D] = value[b, h]
            vs[i] = vf.reshape(NT, 128, D + 1).transpose(1, 0, 2)
        in_maps.append({"qt": qs, "kt": ks, "v": vs})
    return in_maps


def kernel(query, key, value, mask):
    """Full-input attention; shards over 8 NeuronCores internally."""
    global last_results
    query = np.asarray(query, dtype=np.float32)
    key = np.asarray(key, dtype=np.float32)
    value = np.asarray(value, dtype=np.float32)
    mask = np.asarray(mask)

    idx, cnt, SP = _plan(mask)
    nc = _NC_CACHE.get(SP)
    if nc is None:
        nc = _NC_CACHE[SP] = build_attention_nc(NP=PAIRS_PER_CORE, SP=SP)

    in_maps = build_in_maps(query, key, value, idx, cnt, SP)

    trace = os.environ.get("KERNEL_PROFILE", "") == "1"
    if trace:
        _install_profile_hook()
        try:
            import jax

            jax.device_put(
                np.zeros((4,), np.float32), jax.devices()[0]
            ).block_until_ready()
        except Exception as e:
            print(f"profile warmup failed ({e}); disabling trace", file=sys.stderr)
            trace = False
    res = run_bass_kernel_spmd(nc, in_maps, core_ids=list(range(N_CORES)), trace=trace)
    last_results = res

    chunks = _chunk_plan(SP)
    out = np.empty((B, H, S, D), dtype=np.float32)
    for c in range(N_CORES):
        oc = np.asarray(res.results[c]["o"], dtype=np.float32)
        for i in range(PAIRS_PER_CORE):
            pair = c * PAIRS_PER_CORE + i
            b, h = pair // H, pair % H
            full = np.empty((SP, D), dtype=np.float32)
            for ci, (s0, sw) in enumerate(chunks):
                blk = oc[i, :, ci, :]
                full[s0 : s0 + sw] = (blk[0:D, 0:sw] / blk[D, 0:sw]).T
            out[b, h, idx[b]] = full[: cnt[b]]
            if cnt[b] < S:
                out[b, h, np.nonzero(mask[b] == 0)[0]] = full[cnt[b]]
    return out


# revision 21
# speedup vs baseline: 1.2052x; 1.2052x over previous
"""TRN2 Bass kernel for nn_Attention_11252814315826.

out[b,h,s,:] = softmax(Q[b,h] @ K^T[b,h] / 8 + addr(mask)) @ V[b,h]
with the additive mask on the QUERY dim: for mask[b,s]==0 the reference's
-1e12 row offset makes softmax exactly uniform, so out = colmean(V[b,h]).

Strategy (v3): shard the 32 (b,h) pairs 4-per-core across 8 NeuronCores.
Host-side: compact query rows to the mask==1 subset, pre-transpose to
Q^T [128, SP] fp16 with rows 64-127 a duplicate of 0-63, and pack K^T as
[128, 8, 128] fp16 holding key-block pairs (even t in partitions 0-63,
odd t in 64-127).

Device per pair: QK^T runs as TWO CONCURRENT row-tiled 64-contract
matmuls (tile_position rows 0-63 / 64-127) writing the two bank-halves
of one [128, 1024] PSUM tile. This both doubles QK throughput and -- the
key discovery -- keeps the PE HAM un-throttled: matmuls that only drive
64 of the 128 contract rows never reach the "busy" activity threshold
and the PE stays clamped at 1.2 GHz; full-row activity runs at 2.4 GHz.
A ~5us burst of full-contract warmup matmuls during the initial DMA fill
triggers the un-throttle before real work starts.

exp uses the bitcast fast-exp (i16 = score*184.66 + const, reinterpreted
as fp16 == 2^(x*log2e) with linear mantissa interp; the +-3% sawtooth
cancels through softmax normalization only if EVERY key block uses the
identical formula, so both engines run the same math). One fused op per
slot covers both PSUM halves (FD=1024) to amortize the per-op overhead:
wide slots alternate ACT (Copy activation w/ scale+bias) and DVE
(tensor_scalar mult+add); narrow tail slots go to the DVE, per-chunk
epilogues to the ACT, balancing both engines just under the PE pace.

PV accumulates [V|1]^T @ E in PSUM giving numerator and denominator
together. The epilogue copies raw [num|den] to fp16 SBUF and DMAs it
out; f32 divide and [d,s]->[s,d] transpose happen on the host.
"""

import os
import sys

for _p in (
    "/root/.axon_site",
    "/root/.axon_site/_ro/trn_rl_repo",
    "/root/.axon_site/_ro/pypackages",
    "/opt/trn_rl_repo",
):
    if os.path.isdir(_p) and _p not in sys.path:
        sys.path.append(_p)

from concourse.bass_utils import run_bass_kernel_spmd

import numpy as np

import concourse.bacc as bacc
import concourse.tile as tile
import concourse.mybir as mybir

F32 = mybir.dt.float32
F16 = mybir.dt.float16
I16 = mybir.dt.int16

LOG2E = 1.4426950408889634
S0 = 3.0  # exponent shift: exp(x/8 - S0); cancels in softmax, keeps fp16 range
FE_SCALE = 0.125 * 1024 * LOG2E          # 184.66496...
FE_BIAS = 15 * 1024 - S0 * 1024 * LOG2E - 44.0


def _chunk_plan(SP):
    """Split SP query columns into chunks of width <=512 (PSUM bank limit)."""
    chunks = []
    s0 = 0
    while s0 < SP:
        w = min(512, SP - s0)
        chunks.append((s0, w))
        s0 += w
    return chunks


def build_attention_nc(NP=4, SP=1056, S=2048, D=64):
    assert S % 256 == 0 and D == 64 and SP % 32 == 0
    NT = S // 128   # 16 key blocks of 128
    NJ = NT // 2    # 8 packed key-block pairs
    chunks = _chunk_plan(SP)
    NCH = len(chunks)

    nc = bacc.Bacc("TRN2", target_bir_lowering=False, debug=False)

    qt = nc.dram_tensor("qt", [NP, 128, SP], F16, kind="ExternalInput")
    kt = nc.dram_tensor("kt", [NP, 128, NJ, 128], F16, kind="ExternalInput")
    # v pre-arranged on host to the device layout [128, NT, 65] so the
    # load is one big contiguous-row DMA (strided-gather descriptors were
    # serializing the Sync engine's DMA dispatch).
    v = nc.dram_tensor("v", [NP, 128, NT, D + 1], F16, kind="ExternalInput")
    o = nc.dram_tensor("o", [NP, D + 1, NCH, 512], F16, kind="ExternalOutput")

    dma = nc.sync

    debug = os.environ.get("KERNEL_DEBUG", "0") == "1"
    if debug:
        dbg_qk = nc.dram_tensor("dbg_qk", [NJ, 128, 2, 512], F32, kind="ExternalOutput")
        dbg_exp = nc.dram_tensor("dbg_exp", [NJ, 128, 2, 512], F16, kind="ExternalOutput")

    ctxs = {}

    with tile.TileContext(nc) as tc:
        with (
            tc.tile_pool(name="const", bufs=1) as const_pool,
            tc.tile_pool(name="kt", bufs=1) as kt_pool,
            tc.tile_pool(name="qt", bufs=1) as qt_pool,
            tc.tile_pool(name="v", bufs=1) as v_pool,
            tc.tile_pool(name="exp", bufs=6) as exp_pool,
            tc.tile_pool(name="osb", bufs=4) as osb_pool,
            tc.tile_pool(name="qkps", bufs=3, space="PSUM") as qk_psum,
            tc.tile_pool(name="pvps", bufs=2, space="PSUM") as pv_psum,
        ):
            ident = const_pool.tile([128, 128], F16)
            nc.vector.memset(ident[:], 0.0)

            def pe_warmup(n=int(os.environ.get("KERNEL_WARM", "48"))):
                # Full-contract (128-row) matmuls during the initial DMA
                # fill: HAM only counts full-row activity toward the
                # "busy" un-throttle threshold, so these flip the PE to
                # 2.4 GHz before the real stream begins. The zeros tile
                # comes from a DVE memset, not a DMA, so warmup starts at
                # t~0 instead of waiting out the DMA-queue preamble.
                warm_ps = qk_psum.tile([128, 2, 512], F32, tag="qkp", name="warm")
                for i in range(n):
                    nc.tensor.matmul(
                        warm_ps[:, 0, 0:128],
                        ident[:, :],
                        ident[:, :],
                        start=True,
                        stop=True,
                        skip_group_check=True,
                    )
                warm_out = const_pool.tile([128, 8], F32, tag="warmout")
                nc.vector.tensor_copy(warm_out[:], warm_ps[:, 0, 0:8])

            def pair_prologue(p):
                # input DMAs issued once; inputs stay resident in SBUF.
                # One big contiguous DMA per tensor: Sync-engine dispatch
                # (~0.8us per dma_start) was the hidden serial resource.
                qt_sb = qt_pool.tile([128, SP], F16, tag=f"qt{p}", name=f"qt{p}")
                dma.dma_start(qt_sb[:], qt.ap()[p])
                kt_sb = kt_pool.tile([128, NJ, 128], F16, tag=f"kt{p}", name=f"kt{p}")
                dma.dma_start(kt_sb[:], kt.ap()[p])
                v_sb = v_pool.tile([128, NT, D + 1], F16, tag=f"v{p}", name=f"v{p}")
                dma.dma_start(v_sb[:], v.ap()[p])
                ctxs[p] = dict(kt=kt_sb, qt=qt_sb, v=v_sb)

            ballast = os.environ.get("KERNEL_BALLAST", "1") == "1"

            def emit_qk(p, s0, sw, j):
                # two concurrent row-tiled 64-contract matmuls (rows 0-63
                # and 64-127) -> the two bank-halves of one PSUM tile.
                cx = ctxs[p]
                qk_ps = qk_psum.tile([128, 2, 512], F32, tag="qkp")
                nc.tensor.matmul(
                    qk_ps[:, 0, 0:sw],
                    cx["kt"][0:64, j, :],
                    cx["qt"][0:64, s0 : s0 + sw],
                    start=True,
                    stop=True,
                    skip_group_check=True,
                )
                nc.tensor.matmul(
                    qk_ps[:, 1, 0:sw],
                    cx["kt"][64:128, j, :],
                    cx["qt"][64:128, s0 : s0 + sw],
                    start=True,
                    stop=True,
                    skip_group_check=True,
                )
                if ballast and sw <= 128:
                    # narrow tail slots leave the PE mostly idle; HAM then
                    # reads low full-row activity across the 3.4us window and
                    # re-throttles the clock at every pair boundary. A dummy
                    # full-contract matmul into unused PSUM columns of this
                    # same tile keeps the activity monitor fed.
                    nc.tensor.matmul(
                        qk_ps[:, 0, 256:512],
                        ident[:, :],
                        cx["qt"][:, 0:256],
                        start=True,
                        stop=True,
                        skip_group_check=True,
                    )
                return qk_ps

            exp_rot = [0]

            def emit_exp(p, sw, qk_ps, narrow):
                # one fused op over both halves; identical fast-exp math
                # on either engine (sawtooth must match across all key
                # blocks to cancel in normalization).
                # split mode: the slot's two halves go to BOTH engines in
                # parallel -> per-slot exp latency (~690ns) fits inside one
                # PE slot, so lagged PV matmuls never wait on exp. fused
                # mode: one FD=2*sw op on alternating engines -> lower
                # total engine time (overhead amortized) but ~2-slot
                # latency; needs a deeper PV lag.
                exp_sb = exp_pool.tile([128, 2, 512], F16, tag="exp")
                if os.environ.get("KERNEL_EXPMODE", "split") == "fused":
                    eng = 1 if narrow else exp_rot[0] % 2
                    exp_rot[0] += 1
                    plan = [(qk_ps[:, :, 0:sw], exp_sb[:, :, 0:sw].bitcast(I16), eng)]
                else:
                    plan = [
                        (qk_ps[:, h, 0:sw], exp_sb[:, h, 0:sw].bitcast(I16), h)
                        for h in range(2)
                    ]
                for src, dst, eng in plan:
                    if eng == 0:
                        nc.scalar.activation(
                            dst,
                            src,
                            mybir.ActivationFunctionType.Copy,
                            scale=FE_SCALE,
                            bias=FE_BIAS,
                        )
                    else:
                        nc.vector.tensor_scalar(
                            dst,
                            src,
                            FE_SCALE,
                            FE_BIAS,
                            op0=mybir.AluOpType.mult,
                            op1=mybir.AluOpType.add,
                        )
                return exp_sb

            def make_pv(p, sw, j, half, exp_sb, pv_ps):
                t = 2 * j + half

                def emit():
                    v_sb = ctxs[p]["v"]
                    nc.tensor.matmul(
                        pv_ps[:, 0:sw],
                        v_sb[:, t, :],
                        exp_sb[:, half, 0:sw],
                        start=(t == 0),
                        stop=(t == NT - 1),
                        skip_group_check=True,
                    )

                return emit

            def make_epilogue(p, ci, sw, pv_ps, o_sb, last):
                # raw [num | den] into the pair's staging tile; one DMA per
                # pair after its last chunk. Normalization and the
                # [d, s] -> [s, d] transpose happen on the host.
                def emit():
                    if exp_rot[0] % 2 == 0:
                        nc.scalar.activation(
                            o_sb[:, ci, 0:sw],
                            pv_ps[:, 0:sw],
                            mybir.ActivationFunctionType.Copy,
                            scale=1.0,
                        )
                    else:
                        nc.vector.tensor_copy(o_sb[:, ci, 0:sw], pv_ps[:, 0:sw])
                    exp_rot[0] += 1
                    if last:
                        dma.dma_start(o.ap()[p], o_sb[:])

                return emit

            # ---- slot-pipelined emission -----------------------------------
            # Per slot (key-block pair j): packed QK -> fused exp -> two PV
            # matmuls with lag so exp can finish. Epilogues deferred 3 slots.
            def emit_body():
                step = [0]
                pvq = []      # deferred (chunk_uid, fn) PV emitters (lag in slots)
                delayed = []  # (due_step, chunk_uid, fn) epilogues
                lag = 2 * int(os.environ.get("KERNEL_PVLAG", "2"))

                def flush_chunk(uid):
                    # emit any of this chunk's PV matmuls still queued BEFORE
                    # its epilogue reads the PSUM accumulator (front entries
                    # are oldest, so pop from the front).
                    while pvq and pvq[0][0] <= uid:
                        pvq.pop(0)[1]()

                def tick():
                    step[0] += 1
                    for due, uid, fn in [d for d in delayed if d[0] <= step[0]]:
                        delayed.remove((due, uid, fn))
                        flush_chunk(uid)
                        fn()
                    while len(pvq) >= lag:
                        pvq.pop(0)[1]()

                uid = 0
                for p in range(NP):
                    o_sb = osb_pool.tile([D + 1, NCH, 512], F16, tag="osb")
                    for ci, (s0, sw) in enumerate(chunks):
                        uid += 1
                        pv_ps = pv_psum.tile([D + 1, sw], F32, tag="pvp")
                        narrow = sw <= 128
                        for j in range(NJ):
                            tick()
                            qk_ps = emit_qk(p, s0, sw, j)
                            if debug and p == 0 and ci == 0:
                                dqk = osb_pool.tile([128, 2, 512], F32, tag=f"dqk{j}")
                                nc.vector.tensor_copy(dqk[:], qk_ps[:, :, :])
                                dma.dma_start(dbg_qk.ap()[j], dqk[:])
                            exp_sb = emit_exp(p, sw, qk_ps, narrow)
                            if debug and p == 0 and ci == 0:
                                dma.dma_start(dbg_exp.ap()[j], exp_sb[:, :, :])
                            pvq.append((uid, make_pv(p, sw, j, 0, exp_sb, pv_ps)))
                            pvq.append((uid, make_pv(p, sw, j, 1, exp_sb, pv_ps)))
                        delayed.append(
                            (
                                step[0] + 3,
                                uid,
                                make_epilogue(
                                    p, ci, sw, pv_ps, o_sb, ci == NCH - 1
                                ),
                            )
                        )
                while pvq:
                    pvq.pop(0)[1]()
                for _, uid, fn in delayed:
                    fn()

            for p in range(NP):
                pair_prologue(p)
            pe_warmup()
            emit_body()

    nc.compile()
    return nc


B, H = 2, 16
S, D = 2048, 64
N_CORES = 8
PAIRS_PER_CORE = (B * H) // N_CORES  # 4

_NC_CACHE = {}
last_results = None


def _install_profile_hook():
    """Wire up the axon NTFF profiling hook if the image's antenv lacks it."""
    import types

    try:
        import antenv.axon_hooks  # noqa: F401

        return
    except ImportError:
        pass
    try:
        from trn_agent_boot.trn_boot import _ntff_profile_via_ctypes

        hook = _ntff_profile_via_ctypes("/opt/axon/libaxon_pjrt.so")
    except Exception:
        hook = None
    mod = types.ModuleType("antenv.axon_hooks")
    mod._hook = hook
    mod.get_axon_ntff_profile_hook = lambda: mod._hook
    mod.set_axon_ntff_profile_hook = lambda h: setattr(mod, "_hook", h)
    sys.modules["antenv.axon_hooks"] = mod
    import antenv

    antenv.axon_hooks = mod
    import concourse.bass_utils as _bu

    _bu.upload_artifacts = lambda tmpdir: "local://" + tmpdir


def _plan(mask):
    idx = [np.nonzero(mask[b] != 0)[0] for b in range(B)]
    cnt = [len(ix) for ix in idx]
    need = max(c + (1 if c < S else 0) for c in cnt)
    SP = max(128, -(-need // 32) * 32)
    return idx, cnt, SP


def build_in_maps(query, key, value, idx, cnt, SP):
    NJ = S // 256
    NT = S // 128
    in_maps = []
    for c in range(N_CORES):
        qs = np.zeros((PAIRS_PER_CORE, 128, SP), dtype=np.float16)
        ks = np.empty((PAIRS_PER_CORE, 128, NJ, 128), dtype=np.float16)
        vs = np.empty((PAIRS_PER_CORE, 128, NT, D + 1), dtype=np.float16)
        for i in range(PAIRS_PER_CORE):
            pair = c * PAIRS_PER_CORE + i
            b, h = pair // H, pair % H
            qT = query[b, h, idx[b]].T.astype(np.float16)  # [D, cnt]
            qs[i, :D, : cnt[b]] = qT
            qs[i, D:, : cnt[b]] = qT
            kb = key[b, h].astype(np.float16).reshape(D, NJ, 2, 128)
            ks[i, :D] = kb[:, :, 0, :]
            ks[i, D:] = kb[:, :, 1, :]
            vf = np.ones((S, D + 1), dtype=np.float16)
            vf[:, :D] = value[b, h]
            vs[i] = vf.reshape(NT, 128, D + 1).transpose(1, 0, 2)
        in_maps.append({"qt": qs, "kt": ks, "v": vs})
    return in_maps


def kernel(query, key, value, mask):
    """Full-input attention; shards over 8 NeuronCores internally."""
    global last_results
    query = np.asarray(query, dtype=np.float32)
    key = np.asarray(key, dtype=np.float32)
    value = np.asarray(value, dtype=np.float32)
    mask = np.asarray(mask)

    idx, cnt, SP = _plan(mask)
    nc = _NC_CACHE.get(SP)
    if nc is None:
        nc = _NC_CACHE[SP] = build_attention_nc(NP=PAIRS_PER_CORE, SP=SP)

    in_maps = build_in_maps(query, key, value, idx, cnt, SP)

    trace = os.environ.get("KERNEL_PROFILE", "") == "1"
    if trace:
        _install_profile_hook()
        try:
            import jax

            jax.device_put(
                np.zeros((4,), np.float32), jax.devices()[0]
            ).block_until_ready()
        except Exception as e:
            print(f"profile warmup failed ({e}); disabling trace", file=sys.stderr)
            trace = False
    res = run_bass_kernel_spmd(nc, in_maps, core_ids=list(range(N_CORES)), trace=trace)
    last_results = res

    chunks = _chunk_plan(SP)
    out = np.empty((B, H, S, D), dtype=np.float32)
    for c in range(N_CORES):
        oc = np.asarray(res.results[c]["o"], dtype=np.float32)
        for i in range(PAIRS_PER_CORE):
            pair = c * PAIRS_PER_CORE + i
            b, h = pair // H, pair % H
            full = np.empty((SP, D), dtype=np.float32)
            for ci, (s0, sw) in enumerate(chunks):
                blk = oc[i, :, ci, :]
                full[s0 : s0 + sw] = (blk[0:D, 0:sw] / blk[D, 0:sw]).T
            out[b, h, idx[b]] = full[: cnt[b]]
            if cnt[b] < S:
                out[b, h, np.nonzero(mask[b] == 0)[0]] = full[cnt[b]]
    return out
